# revision 28
# baseline (speedup 1.0000x reference)
"""Trainium2 Bass kernel for nn_Divergence2d.

Math (from the reference):
  q = C//4 = 4 channel groups A=x[:, :4], B=x[:,4:8], C=x[:,8:12], D=x[:,12:16]
  With per-group channel sums  Asum(r,c) = sum_ch lam_ch x[ch, r, c]  (lam only
  for group A):

    out1[i,j] = lam*(Asum[i-1, j] - Asum[i-1, j-2]) + Bsum[i-2, j-1] - Bsum[i, j-1]
    out2[i,j] =     (Csum[i-1, j] - Csum[i-1, j-2]) + Dsum[i-2, j-1] - Dsum[i, j-1]

  for i,j in [0, 514), with zero padding outside [0,512).

Strategy (v5, pure data parallel, 2 images per core on 8 cores):
  The op is memory-bound: 38 MB HBM traffic/core across 19k 2 KB DMA
  descriptors is a ~125 us floor at the measured per-engine packet rate.
  Every design choice below exists to keep the 16 DMA engines streaming:

  - per 126-row block, a 128-row window of all 16 channels is loaded into
    [128 rows, 16ch x 512].  Descriptor order [row, ch] makes DMA engine c
    stream channel c's rows sequentially from HBM (HBM-friendly).  The load
    is split into two row-halves issued on different queues (SP + Pool
    SWDGE) because a HWDGE dma_start occupies its queue for the whole
    transfer - one queue would serialize input delivery at ~11 us/block.
  - DVE (fp32): channel sums via grouped adds (3D APs), then the
    *horizontal* stencil diff, writing bf16 maps
    (hA = Asum[:, j] - Asum[:, j-2]; hB = Bsum[:, j-1] via padded layout).
    DVE instruction cost is ~1.08 ns per free-dim element regardless of
    partition count, so ops are organized as few wide-partition passes.
  - TensorE (bf16, full rate): the *vertical* shifts as one-hot shift
    matmuls, with the final combine done by PSUM accumulation:
    psOut1 = (lam*S1)@hA + Sbd@hB (start/stop pairs kept contiguous -
    interleaving accumulation groups corrupts results).  Compute-engine
    APs cannot start at a nonzero partition on TRN2, so row shifts must
    go through the PE.  Image-boundary zero padding is folded into
    per-block weight variants (top block shifts by -1, tail block masks
    out-of-range rows) so no window/map memsets are needed at all.
  - ACT drains PSUM straight into the output tile; one store per block on
    the ACT queue (dependency-aligned with the drains).

  Only the maps are bf16-rounded (weights are exact 0/+-lam/+-1), giving
  ~1.7e-3 l2 rel error vs the 2e-2 gate.
"""
import sys

for _p in (
    "/root/.axon_site",
    "/root/.axon_site/_ro/trn_rl_repo",
    "/root/.axon_site/_ro/pypackages",
    "/opt/trn_rl_repo",
):
    if _p not in sys.path:
        sys.path.append(_p)

import numpy as np

N_CORES = 8
N, C, H, W = 16, 16, 512, 512
PB = N // N_CORES          # images per core
HO = WO = H + 2            # 514
BLK = 126                  # output rows per block
BLOCKS = []
_i0 = 0
while _i0 < HO:
    BLOCKS.append((_i0, min(BLK, HO - _i0)))
    _i0 += BLK
# -> [(0,126), (126,126), (252,126), (378,126), (504,10)]

_cache = {}


def _build(lam4):
    import concourse.bacc as bacc
    import concourse.mybir as mybir
    from concourse.tile import TileContext

    f32 = mybir.dt.float32
    bf16 = mybir.dt.bfloat16
    ALU = mybir.AluOpType
    ACT_COPY = mybir.ActivationFunctionType.Copy
    lam_eq = all(float(v) == float(lam4[0]) for v in lam4)
    lam0 = float(lam4[0])

    nc = bacc.Bacc("TRN2", target_bir_lowering=False, debug=False,
                   num_devices=N_CORES, detect_race_conditions=False)
    x = nc.dram_tensor("x", (PB, C, H, W), f32, kind="ExternalInput")
    out = nc.dram_tensor("out", (PB, 2, HO, WO), f32, kind="ExternalOutput")

    with TileContext(nc) as tc:
        with (
            tc.tile_pool(name="consts", bufs=1) as c_pool,
            tc.tile_pool(name="rhs", bufs=5) as rhs_pool,
            tc.tile_pool(name="work", bufs=1) as w_pool,
            tc.tile_pool(name="hmaps", bufs=2) as h_pool,
            tc.tile_pool(name="psum", bufs=2, space="PSUM") as ps_pool,
            tc.tile_pool(name="outs", bufs=3) as out_pool,
        ):
            # ---- one-time shift weights [128 window rows, BLK out rows] ----
            # window row w holds x row rlo+w; out row i0+m needs x rows
            # i0+m-1 (A/C) and i0+m-2, i0+m (B/D), i.e. w = m+off+k for
            # k in {-1} / {-2, 0} with off = i0-rlo.  Out-of-image rows are
            # simply not selected (top block: off=0; tail: mask w > wmax).
            with tc.tile_pool(name="scratch", bufs=1) as sc_pool:
                R2 = sc_pool.tile([128, BLK], f32, tag="R2")     # w + 2
                nc.gpsimd.iota(R2[:, :], pattern=[[0, BLK]], base=2,
                               channel_multiplier=1,
                               allow_small_or_imprecise_dtypes=True)
                Sm = {}
                for b in (-2, -1, 0, 1, 2):                  # m + b + 2 >= 0
                    t_ = sc_pool.tile([128, BLK], f32, tag=f"Sm{b}",
                                      name=f"Sm{b}")
                    nc.gpsimd.iota(t_[:, :], pattern=[[1, BLK]], base=b + 2,
                                   channel_multiplier=0,
                                   allow_small_or_imprecise_dtypes=True)
                    Sm[b] = t_
                e = {}
                for b in (-2, -1, 0, 1, 2):
                    t_ = sc_pool.tile([128, BLK], f32, tag=f"e{b}",
                                      name=f"e{b}")
                    nc.vector.tensor_tensor(t_[:, :], R2[:, :], Sm[b][:, :],
                                            ALU.is_equal)
                    e[b] = t_

                def mk(tag, pos, neg=None, scale=1.0, mask=None):
                    t_ = c_pool.tile([128, BLK], bf16, tag=tag, name=tag)
                    if neg is None:
                        nc.vector.tensor_scalar_mul(t_[:, :], e[pos][:, :],
                                                    scale)
                    else:
                        nc.vector.tensor_tensor(t_[:, :], e[pos][:, :],
                                                e[neg][:, :], ALU.subtract)
                    if mask is not None:
                        nc.vector.tensor_tensor(t_[:, :], t_[:, :],
                                                mask[:, :], ALU.mult)
                    return t_

                lamw = lam0 if lam_eq else 1.0
                # interior blocks (off=2): A w=m+1; B w=m (+) / m+2 (-)
                SlamI = mk("SlamI", 1, scale=lamw)
                S1I = mk("S1I", 1)
                SbdI = mk("SbdI", 0, 2)
                # top block (off=0): A w=m-1; B w=m-2 (+) / m (-)
                SlamT = mk("SlamT", -1, scale=lamw)
                S1T = mk("S1T", -1)
                SbdT = mk("SbdT", -2, 0)
                # tail block (off=2, only w<=9 valid): masked interior weights
                msk = sc_pool.tile([128, BLK], f32, tag="msk")
                nc.vector.tensor_scalar(msk[:, :], R2[:, :], 11.5, None,
                                        ALU.is_lt)
                SlamZ = mk("SlamZ", 1, scale=lamw, mask=msk)
                S1Z = mk("S1Z", 1, mask=msk)
                SbdZ = mk("SbdZ", 0, 2, mask=msk)

            # ---- work tiles: s1/mpAC are DVE-private (bufs=1), hAC/hBD are
            # read by the PE so they rotate over 2 buffers (otherwise the
            # next block's DVE writes stall on the previous block's matmuls)
            # bf16 intermediates: DVE 16-bit ops run ~2x, and the PE needs
            # bf16 operands anyway; costs ~1 extra rounding step per map
            s1 = w_pool.tile([128, 4 * 1024], bf16, tag="s1")
            mpAC = w_pool.tile([128, 2 * 516], bf16, tag="mpAC")
            if not lam_eq:
                tA = w_pool.tile([128, 4 * 512], f32, tag="tA")
            s1v = s1[:, :].rearrange("p (g k) -> p g k", k=1024)
            mpv = mpAC[:, :].rearrange("p (m c) -> p m c", c=516)
            nc.vector.memset(mpAC[:, :], 0.0)

            # zero both rotating buffers once: covers the column pads (data
            # ops never write them: mpAC data cols [2,514), hBD [1,513)) and
            # makes the never-selected stale rows of the first blocks finite
            for _ in range(2):
                hAC = h_pool.tile([128, 2 * WO], bf16, tag="hAC")
                hBD = h_pool.tile([128, 2 * WO], bf16, tag="hBD")
                nc.vector.memset(hAC[:, :], 0.0)
                nc.vector.memset(hBD[:, :], 0.0)

            # ---- main loop ---------------------------------------------
            for n in range(PB):
                for bi, (i0, nr) in enumerate(BLOCKS):
                    rlo = max(i0 - 2, 0)
                    rhi = min(i0 + nr, H)
                    P = rhi - rlo               # valid window rows
                    if bi == 0:
                        Sl, S1_, Sb = SlamT, S1T, SbdT
                    elif rhi == H and i0 + nr > H:
                        Sl, S1_, Sb = SlamZ, S1Z, SbdZ
                    else:
                        Sl, S1_, Sb = SlamI, S1I, SbdI
                    t = rhs_pool.tile([128, 16 * 512], f32, tag="rhs")
                    tv = t[:, :].rearrange("p (c w) -> p c w", w=512)
                    # ONE dma_start per block on the SP queue: [row, ch]
                    # descriptor order keeps DMA engine c streaming channel
                    # c's rows sequentially from HBM.  A single pure stream
                    # measures ~107 ns/2KB packet; splitting the load across
                    # queues (by channel, row, or via SWDGE) interleaves
                    # streams at the engines and degrades packets 25-60%.
                    nc.sync.dma_start(out=tv[0:P, :, :],
                                      in_=x[n, :, rlo:rhi, :].rearrange(
                                          "c r w -> r c w"))

                    hAC = h_pool.tile([128, 2 * WO], bf16, tag="hAC")
                    hBD = h_pool.tile([128, 2 * WO], bf16, tag="hBD")
                    hBDv = hBD[:, :].rearrange("p (m c) -> p m c", c=WO)
                    tg = t[:, :].rearrange("p (g k) -> p g k", k=2048)
                    # -- channel sums (fp32, all on DVE: other engines
                    #    contend for SBUF ports and slow everything down) --
                    if lam_eq:
                        nc.vector.tensor_tensor(
                            s1[0:P, :], tg[0:P, 0:4, 0:1024],
                            tg[0:P, 0:4, 1024:2048], ALU.add)
                    else:
                        tAv = tA[:, :].rearrange("p (c w) -> p c w", w=512)
                        for c4 in range(4):
                            nc.vector.tensor_scalar_mul(
                                tAv[0:P, c4, :], tv[0:P, c4, :], float(lam4[c4]))
                        nc.vector.tensor_tensor(
                            s1[0:P, 0:1024], tA[0:P, 0:1024],
                            tA[0:P, 1024:2048], ALU.add)
                        nc.vector.tensor_tensor(
                            s1v[0:P, 1:4, :], tg[0:P, 1:4, 0:1024],
                            tg[0:P, 1:4, 1024:2048], ALU.add)
                    # A,C sums into padded fp32 maps (s1 groups 0,2)
                    nc.vector.tensor_tensor(
                        mpv[0:P, 0:2, 2:514], s1v[0:P, 0:3:2, 0:512],
                        s1v[0:P, 0:3:2, 512:1024], ALU.add)
                    # horizontal diff -> bf16: hA[p,j] = Asum[p,j]-Asum[p,j-2]
                    hACv = hAC[:, :].rearrange("p (m c) -> p m c", c=WO)
                    nc.vector.tensor_tensor(
                        hACv[0:P, 0:2, :], mpv[0:P, 0:2, 2:516],
                        mpv[0:P, 0:2, 0:514], ALU.subtract)
                    # B,D sums straight into padded bf16 maps (s1 groups 1,3)
                    nc.vector.tensor_tensor(
                        hBDv[0:P, 0:2, 1:513], s1v[0:P, 1:4:2, 0:512],
                        s1v[0:P, 1:4:2, 512:1024], ALU.add)

                    # -- vertical shifts + combine on the PE: both stencil
                    #    terms accumulate into the same PSUM region; each
                    #    start->stop pair kept contiguous in issue order
                    #    (interleaved accumulation groups corrupt) --
                    # one 3-bank PSUM tile: [out1-int | out2-int | 4 edge cols]
                    psO = ps_pool.tile([128, 1032], f32, tag="psO", name="psO")
                    nc.tensor.matmul(psO[0:nr, 0:512], Sl[:, 0:nr],
                                     hAC[:, 0:512], start=True, stop=False)
                    nc.tensor.matmul(psO[0:nr, 0:512], Sb[:, 0:nr],
                                     hBD[:, 0:512], start=False, stop=True)
                    nc.tensor.matmul(psO[0:nr, 1024:1026], Sl[:, 0:nr],
                                     hAC[:, 512:514], start=True, stop=False)
                    nc.tensor.matmul(psO[0:nr, 1024:1026], Sb[:, 0:nr],
                                     hBD[:, 512:514], start=False, stop=True)
                    nc.tensor.matmul(psO[0:nr, 512:1024], S1_[:, 0:nr],
                                     hAC[:, WO:WO + 512], start=True, stop=False)
                    nc.tensor.matmul(psO[0:nr, 512:1024], Sb[:, 0:nr],
                                     hBD[:, WO:WO + 512], start=False, stop=True)
                    nc.tensor.matmul(psO[0:nr, 1026:1028], S1_[:, 0:nr],
                                     hAC[:, WO + 512:WO + 514],
                                     start=True, stop=False)
                    nc.tensor.matmul(psO[0:nr, 1026:1028], Sb[:, 0:nr],
                                     hBD[:, WO + 512:WO + 514],
                                     start=False, stop=True)

                    # -- ACT drains PSUM straight into the output tile
                    #    (two strided ops instead of four: fewer sem hops) --
                    o = out_pool.tile([128, 2 * WO], f32, tag="o")
                    ov2 = o[0:nr, :].rearrange("p (ch w) -> p ch w", w=WO)
                    psv = psO[0:nr, 0:1024].rearrange("p (h w) -> p h w", w=512)
                    nc.scalar.activation(ov2[:, 0:2, 0:512], psv[:, 0:2, :],
                                         ACT_COPY)
                    nc.scalar.activation(
                        ov2[:, 0:2, 512:514],
                        psO[0:nr, 1024:1028].rearrange(
                            "p (h w) -> p h w", w=2), ACT_COPY)
                    osrc = o[0:nr, :].rearrange("p (ch w) -> p ch w", w=WO)
                    ov = out[n].rearrange("ch r w -> r ch w")
                    nc.scalar.dma_start(out=ov[i0:i0 + nr, :, :], in_=osrc)
    nc.finalize()
    return nc


def _get_nc(lam4):
    key = tuple(float(v) for v in lam4)
    if key not in _cache:
        _cache[key] = _build(key)
    return _cache[key]


def _run(xs: np.ndarray, lam4, trace: bool = False, tmpdir=None):
    from concourse.bass_utils import run_bass_kernel_spmd

    nc = _get_nc(lam4)
    in_maps = [{"x": np.ascontiguousarray(xs[PB * c:PB * (c + 1)])}
               for c in range(N_CORES)]
    res = run_bass_kernel_spmd(nc, in_maps, list(range(N_CORES)),
                               trace=trace, tmpdir=tmpdir)
    full = np.concatenate([res.results[c]["out"] for c in range(N_CORES)], axis=0)
    return full, res


def kernel(x, lam1x, lam2x, lam1y, lam2y):
    x = np.ascontiguousarray(np.asarray(x, dtype=np.float32))
    assert x.shape == (N, C, H, W), x.shape
    lam4 = np.asarray(lam1x, dtype=np.float32).reshape(-1)
    assert lam4.shape == (4,), lam4.shape
    full, _ = _run(x, lam4)
    return full


# revision 29
# speedup vs baseline: 1.0271x; 1.0271x over previous
"""Trainium2 Bass kernel for nn_Divergence2d.

Math (from the reference):
  q = C//4 = 4 channel groups A=x[:, :4], B=x[:,4:8], C=x[:,8:12], D=x[:,12:16]
  With per-group channel sums  Asum(r,c) = sum_ch lam_ch x[ch, r, c]  (lam only
  for group A):

    out1[i,j] = lam*(Asum[i-1, j] - Asum[i-1, j-2]) + Bsum[i-2, j-1] - Bsum[i, j-1]
    out2[i,j] =     (Csum[i-1, j] - Csum[i-1, j-2]) + Dsum[i-2, j-1] - Dsum[i, j-1]

  for i,j in [0, 514), with zero padding outside [0,512).

Strategy (v5, pure data parallel, 2 images per core on 8 cores):
  The op is memory-bound: 38 MB HBM traffic/core across 19k 2 KB DMA
  descriptors is a ~125 us floor at the measured per-engine packet rate.
  Every design choice below exists to keep the 16 DMA engines streaming:

  - per 126-row block, a 128-row window of all 16 channels is loaded into
    [128 rows, 16ch x 512].  Descriptor order [row, ch] makes DMA engine c
    stream channel c's rows sequentially from HBM (HBM-friendly).  The load
    is split into two row-halves issued on different queues (SP + Pool
    SWDGE) because a HWDGE dma_start occupies its queue for the whole
    transfer - one queue would serialize input delivery at ~11 us/block.
  - DVE (fp32): channel sums via grouped adds (3D APs), then the
    *horizontal* stencil diff, writing bf16 maps
    (hA = Asum[:, j] - Asum[:, j-2]; hB = Bsum[:, j-1] via padded layout).
    DVE instruction cost is ~1.08 ns per free-dim element regardless of
    partition count, so ops are organized as few wide-partition passes.
  - TensorE (bf16, full rate): the *vertical* shifts as one-hot shift
    matmuls, with the final combine done by PSUM accumulation:
    psOut1 = (lam*S1)@hA + Sbd@hB (start/stop pairs kept contiguous -
    interleaving accumulation groups corrupts results).  Compute-engine
    APs cannot start at a nonzero partition on TRN2, so row shifts must
    go through the PE.  Image-boundary zero padding is folded into
    per-block weight variants (top block shifts by -1, tail block masks
    out-of-range rows) so no window/map memsets are needed at all.
  - ACT drains PSUM straight into the output tile; one store per block on
    the ACT queue (dependency-aligned with the drains).

  Only the maps are bf16-rounded (weights are exact 0/+-lam/+-1), giving
  ~1.7e-3 l2 rel error vs the 2e-2 gate.
"""
import sys

for _p in (
    "/root/.axon_site",
    "/root/.axon_site/_ro/trn_rl_repo",
    "/root/.axon_site/_ro/pypackages",
    "/opt/trn_rl_repo",
):
    if _p not in sys.path:
        sys.path.append(_p)

import numpy as np

N_CORES = 8
N, C, H, W = 16, 16, 512, 512
PB = N // N_CORES          # images per core
HO = WO = H + 2            # 514
BLK = 126                  # output rows per block
BLOCKS = []
_i0 = 0
while _i0 < HO:
    BLOCKS.append((_i0, min(BLK, HO - _i0)))
    _i0 += BLK
# -> [(0,126), (126,126), (252,126), (378,126), (504,10)]

_cache = {}


def _build(lam4):
    import concourse.bacc as bacc
    import concourse.mybir as mybir
    from concourse.tile import TileContext

    f32 = mybir.dt.float32
    bf16 = mybir.dt.bfloat16
    ALU = mybir.AluOpType
    ACT_COPY = mybir.ActivationFunctionType.Copy
    lam_eq = all(float(v) == float(lam4[0]) for v in lam4)
    lam0 = float(lam4[0])

    nc = bacc.Bacc("TRN2", target_bir_lowering=False, debug=False,
                   num_devices=N_CORES, detect_race_conditions=False)
    x = nc.dram_tensor("x", (PB, C, H, W), f32, kind="ExternalInput")
    out = nc.dram_tensor("out", (PB, 2, HO, WO), f32, kind="ExternalOutput")

    with TileContext(nc) as tc:
        with (
            tc.tile_pool(name="consts", bufs=1) as c_pool,
            tc.tile_pool(name="rhs", bufs=5) as rhs_pool,
            tc.tile_pool(name="work", bufs=1) as w_pool,
            tc.tile_pool(name="hmaps", bufs=2) as h_pool,
            tc.tile_pool(name="psum", bufs=2, space="PSUM") as ps_pool,
            tc.tile_pool(name="outs", bufs=3) as out_pool,
        ):
            # ---- one-time shift weights [128 window rows, BLK out rows] ----
            # window row w holds x row rlo+w; out row i0+m needs x rows
            # i0+m-1 (A/C) and i0+m-2, i0+m (B/D), i.e. w = m+off+k for
            # k in {-1} / {-2, 0} with off = i0-rlo.  Out-of-image rows are
            # simply not selected (top block: off=0; tail: mask w > wmax).
            with tc.tile_pool(name="scratch", bufs=1) as sc_pool:
                R2 = sc_pool.tile([128, BLK], f32, tag="R2")     # w + 2
                nc.gpsimd.iota(R2[:, :], pattern=[[0, BLK]], base=2,
                               channel_multiplier=1,
                               allow_small_or_imprecise_dtypes=True)
                Sm = {}
                for b in (-2, -1, 0, 1, 2):                  # m + b + 2 >= 0
                    t_ = sc_pool.tile([128, BLK], f32, tag=f"Sm{b}",
                                      name=f"Sm{b}")
                    nc.gpsimd.iota(t_[:, :], pattern=[[1, BLK]], base=b + 2,
                                   channel_multiplier=0,
                                   allow_small_or_imprecise_dtypes=True)
                    Sm[b] = t_
                e = {}
                for b in (-2, -1, 0, 1, 2):
                    t_ = sc_pool.tile([128, BLK], f32, tag=f"e{b}",
                                      name=f"e{b}")
                    nc.vector.tensor_tensor(t_[:, :], R2[:, :], Sm[b][:, :],
                                            ALU.is_equal)
                    e[b] = t_

                def mk(tag, pos, neg=None, scale=1.0, mask=None):
                    t_ = c_pool.tile([128, BLK], bf16, tag=tag, name=tag)
                    if neg is None:
                        nc.vector.tensor_scalar_mul(t_[:, :], e[pos][:, :],
                                                    scale)
                    else:
                        nc.vector.tensor_tensor(t_[:, :], e[pos][:, :],
                                                e[neg][:, :], ALU.subtract)
                    if mask is not None:
                        nc.vector.tensor_tensor(t_[:, :], t_[:, :],
                                                mask[:, :], ALU.mult)
                    return t_

                lamw = lam0 if lam_eq else 1.0
                # interior blocks (off=2): A w=m+1; B w=m (+) / m+2 (-)
                SlamI = mk("SlamI", 1, scale=lamw)
                S1I = mk("S1I", 1)
                SbdI = mk("SbdI", 0, 2)
                # top block (off=0): A w=m-1; B w=m-2 (+) / m (-)
                SlamT = mk("SlamT", -1, scale=lamw)
                S1T = mk("S1T", -1)
                SbdT = mk("SbdT", -2, 0)
                # tail block (off=2, only w<=9 valid): masked interior weights
                msk = sc_pool.tile([128, BLK], f32, tag="msk")
                nc.vector.tensor_scalar(msk[:, :], R2[:, :], 11.5, None,
                                        ALU.is_lt)
                SlamZ = mk("SlamZ", 1, scale=lamw, mask=msk)
                S1Z = mk("S1Z", 1, mask=msk)
                SbdZ = mk("SbdZ", 0, 2, mask=msk)

            # ---- work tiles: s1/mpAC are DVE-private (bufs=1), hAC/hBD are
            # read by the PE so they rotate over 2 buffers (otherwise the
            # next block's DVE writes stall on the previous block's matmuls)
            # bf16 intermediates: DVE 16-bit ops run ~2x, and the PE needs
            # bf16 operands anyway; costs ~1 extra rounding step per map
            s1 = w_pool.tile([128, 4 * 1024], bf16, tag="s1")
            mpAC = w_pool.tile([128, 2 * 516], bf16, tag="mpAC")
            if not lam_eq:
                tA = w_pool.tile([128, 4 * 512], f32, tag="tA")
            s1v = s1[:, :].rearrange("p (g k) -> p g k", k=1024)
            mpv = mpAC[:, :].rearrange("p (m c) -> p m c", c=516)
            nc.vector.memset(mpAC[:, :], 0.0)

            # zero both rotating buffers once: covers the column pads (data
            # ops never write them: mpAC data cols [2,514), hBD [1,513)) and
            # makes the never-selected stale rows of the first blocks finite
            for _ in range(2):
                hAC = h_pool.tile([128, 2 * WO], bf16, tag="hAC")
                hBD = h_pool.tile([128, 2 * WO], bf16, tag="hBD")
                nc.vector.memset(hAC[:, :], 0.0)
                nc.vector.memset(hBD[:, :], 0.0)

            # ---- main loop ---------------------------------------------
            for n in range(PB):
                for bi, (i0, nr) in enumerate(BLOCKS):
                    rlo = max(i0 - 2, 0)
                    rhi = min(i0 + nr, H)
                    P = rhi - rlo               # valid window rows
                    if bi == 0:
                        Sl, S1_, Sb = SlamT, S1T, SbdT
                    elif rhi == H and i0 + nr > H:
                        Sl, S1_, Sb = SlamZ, S1Z, SbdZ
                    else:
                        Sl, S1_, Sb = SlamI, S1I, SbdI
                    t = rhs_pool.tile([128, 16 * 512], f32, tag="rhs")
                    tv = t[:, :].rearrange("p (c w) -> p c w", w=512)
                    # ONE dma_start per block on the SP queue: [row, ch]
                    # descriptor order keeps DMA engine c streaming channel
                    # c's rows sequentially from HBM.  A single pure stream
                    # measures ~107 ns/2KB packet; splitting the load across
                    # queues (by channel, row, or via SWDGE) interleaves
                    # streams at the engines and degrades packets 25-60%.
                    nc.sync.dma_start(out=tv[0:P, :, :],
                                      in_=x[n, :, rlo:rhi, :].rearrange(
                                          "c r w -> r c w"))

                    hAC = h_pool.tile([128, 2 * WO], bf16, tag="hAC")
                    hBD = h_pool.tile([128, 2 * WO], bf16, tag="hBD")
                    hBDv = hBD[:, :].rearrange("p (m c) -> p m c", c=WO)
                    tg = t[:, :].rearrange("p (g k) -> p g k", k=2048)
                    # -- channel sums (fp32, all on DVE: other engines
                    #    contend for SBUF ports and slow everything down) --
                    if lam_eq:
                        nc.vector.tensor_tensor(
                            s1[0:P, :], tg[0:P, 0:4, 0:1024],
                            tg[0:P, 0:4, 1024:2048], ALU.add)
                    else:
                        tAv = tA[:, :].rearrange("p (c w) -> p c w", w=512)
                        for c4 in range(4):
                            nc.vector.tensor_scalar_mul(
                                tAv[0:P, c4, :], tv[0:P, c4, :], float(lam4[c4]))
                        nc.vector.tensor_tensor(
                            s1[0:P, 0:1024], tA[0:P, 0:1024],
                            tA[0:P, 1024:2048], ALU.add)
                        nc.vector.tensor_tensor(
                            s1v[0:P, 1:4, :], tg[0:P, 1:4, 0:1024],
                            tg[0:P, 1:4, 1024:2048], ALU.add)
                    # A,C sums into padded fp32 maps (s1 groups 0,2)
                    nc.vector.tensor_tensor(
                        mpv[0:P, 0:2, 2:514], s1v[0:P, 0:3:2, 0:512],
                        s1v[0:P, 0:3:2, 512:1024], ALU.add)
                    # horizontal diff -> bf16: hA[p,j] = Asum[p,j]-Asum[p,j-2]
                    hACv = hAC[:, :].rearrange("p (m c) -> p m c", c=WO)
                    nc.vector.tensor_tensor(
                        hACv[0:P, 0:2, :], mpv[0:P, 0:2, 2:516],
                        mpv[0:P, 0:2, 0:514], ALU.subtract)
                    # B,D sums straight into padded bf16 maps (s1 groups 1,3)
                    nc.vector.tensor_tensor(
                        hBDv[0:P, 0:2, 1:513], s1v[0:P, 1:4:2, 0:512],
                        s1v[0:P, 1:4:2, 512:1024], ALU.add)

                    # -- vertical shifts + combine on the PE: both stencil
                    #    terms accumulate into the same PSUM region; each
                    #    start->stop pair kept contiguous in issue order
                    #    (interleaved accumulation groups corrupt) --
                    psO1 = ps_pool.tile([128, 512], f32, tag="psO1", name="psO1")
                    psO2 = ps_pool.tile([128, 512], f32, tag="psO2", name="psO2")
                    psE = ps_pool.tile([128, 4], f32, tag="psE", name="psE")
                    nc.tensor.matmul(psO1[0:nr, :], Sl[:, 0:nr],
                                     hAC[:, 0:512], start=True, stop=False)
                    nc.tensor.matmul(psO1[0:nr, :], Sb[:, 0:nr],
                                     hBD[:, 0:512], start=False, stop=True)
                    nc.tensor.matmul(psE[0:nr, 0:2], Sl[:, 0:nr],
                                     hAC[:, 512:514], start=True, stop=False)
                    nc.tensor.matmul(psE[0:nr, 0:2], Sb[:, 0:nr],
                                     hBD[:, 512:514], start=False, stop=True)
                    nc.tensor.matmul(psO2[0:nr, :], S1_[:, 0:nr],
                                     hAC[:, WO:WO + 512], start=True, stop=False)
                    nc.tensor.matmul(psO2[0:nr, :], Sb[:, 0:nr],
                                     hBD[:, WO:WO + 512], start=False, stop=True)
                    nc.tensor.matmul(psE[0:nr, 2:4], S1_[:, 0:nr],
                                     hAC[:, WO + 512:WO + 514],
                                     start=True, stop=False)
                    nc.tensor.matmul(psE[0:nr, 2:4], Sb[:, 0:nr],
                                     hBD[:, WO + 512:WO + 514],
                                     start=False, stop=True)

                    # -- ACT drains PSUM straight into the output tile --
                    o = out_pool.tile([128, 2 * WO], f32, tag="o")
                    nc.scalar.activation(o[0:nr, 0:512], psO1[0:nr, :],
                                         ACT_COPY)
                    nc.scalar.activation(o[0:nr, 512:514], psE[0:nr, 0:2],
                                         ACT_COPY)
                    nc.scalar.activation(o[0:nr, WO:WO + 512], psO2[0:nr, :],
                                         ACT_COPY)
                    nc.scalar.activation(o[0:nr, WO + 512:2 * WO],
                                         psE[0:nr, 2:4], ACT_COPY)
                    osrc = o[0:nr, :].rearrange("p (ch w) -> p ch w", w=WO)
                    ov = out[n].rearrange("ch r w -> r ch w")
                    nc.scalar.dma_start(out=ov[i0:i0 + nr, :, :], in_=osrc)
    nc.finalize()
    return nc


def _get_nc(lam4):
    key = tuple(float(v) for v in lam4)
    if key not in _cache:
        _cache[key] = _build(key)
    return _cache[key]


def _run(xs: np.ndarray, lam4, trace: bool = False, tmpdir=None):
    from concourse.bass_utils import run_bass_kernel_spmd

    nc = _get_nc(lam4)
    in_maps = [{"x": np.ascontiguousarray(xs[PB * c:PB * (c + 1)])}
               for c in range(N_CORES)]
    res = run_bass_kernel_spmd(nc, in_maps, list(range(N_CORES)),
                               trace=trace, tmpdir=tmpdir)
    full = np.concatenate([res.results[c]["out"] for c in range(N_CORES)], axis=0)
    return full, res


def kernel(x, lam1x, lam2x, lam1y, lam2y):
    x = np.ascontiguousarray(np.asarray(x, dtype=np.float32))
    assert x.shape == (N, C, H, W), x.shape
    lam4 = np.asarray(lam1x, dtype=np.float32).reshape(-1)
    assert lam4.shape == (4,), lam4.shape
    full, _ = _run(x, lam4)
    return full


# revision 30
# speedup vs baseline: 1.0282x; 1.0011x over previous
"""Trainium2 Bass kernel for nn_Divergence2d.

Math (from the reference):
  q = C//4 = 4 channel groups A=x[:, :4], B=x[:,4:8], C=x[:,8:12], D=x[:,12:16]
  With per-group channel sums  Asum(r,c) = sum_ch lam_ch x[ch, r, c]  (lam only
  for group A):

    out1[i,j] = lam*(Asum[i-1, j] - Asum[i-1, j-2]) + Bsum[i-2, j-1] - Bsum[i, j-1]
    out2[i,j] =     (Csum[i-1, j] - Csum[i-1, j-2]) + Dsum[i-2, j-1] - Dsum[i, j-1]

  for i,j in [0, 514), with zero padding outside [0,512).

Strategy (v5, pure data parallel, 2 images per core on 8 cores):
  The op is memory-bound: 38 MB HBM traffic/core across 19k 2 KB DMA
  descriptors is a ~125 us floor at the measured per-engine packet rate.
  Every design choice below exists to keep the 16 DMA engines streaming:

  - per 126-row block, a 128-row window of all 16 channels is loaded into
    [128 rows, 16ch x 512].  Descriptor order [row, ch] makes DMA engine c
    stream channel c's rows sequentially from HBM (HBM-friendly).  The load
    is split into two row-halves issued on different queues (SP + Pool
    SWDGE) because a HWDGE dma_start occupies its queue for the whole
    transfer - one queue would serialize input delivery at ~11 us/block.
  - DVE (fp32): channel sums via grouped adds (3D APs), then the
    *horizontal* stencil diff, writing bf16 maps
    (hA = Asum[:, j] - Asum[:, j-2]; hB = Bsum[:, j-1] via padded layout).
    DVE instruction cost is ~1.08 ns per free-dim element regardless of
    partition count, so ops are organized as few wide-partition passes.
  - TensorE (bf16, full rate): the *vertical* shifts as one-hot shift
    matmuls, with the final combine done by PSUM accumulation:
    psOut1 = (lam*S1)@hA + Sbd@hB (start/stop pairs kept contiguous -
    interleaving accumulation groups corrupts results).  Compute-engine
    APs cannot start at a nonzero partition on TRN2, so row shifts must
    go through the PE.  Image-boundary zero padding is folded into
    per-block weight variants (top block shifts by -1, tail block masks
    out-of-range rows) so no window/map memsets are needed at all.
  - ACT drains PSUM straight into the output tile; one store per block on
    the ACT queue (dependency-aligned with the drains).

  Only the maps are bf16-rounded (weights are exact 0/+-lam/+-1), giving
  ~1.7e-3 l2 rel error vs the 2e-2 gate.
"""
import sys

for _p in (
    "/root/.axon_site",
    "/root/.axon_site/_ro/trn_rl_repo",
    "/root/.axon_site/_ro/pypackages",
    "/opt/trn_rl_repo",
):
    if _p not in sys.path:
        sys.path.append(_p)

import numpy as np

N_CORES = 8
N, C, H, W = 16, 16, 512, 512
PB = N // N_CORES          # images per core
HO = WO = H + 2            # 514
BLK = 126                  # output rows per block
BLOCKS = []
_i0 = 0
while _i0 < HO:
    BLOCKS.append((_i0, min(BLK, HO - _i0)))
    _i0 += BLK
# -> [(0,126), (126,126), (252,126), (378,126), (504,10)]

_cache = {}


def _build(lam4):
    import concourse.bacc as bacc
    import concourse.mybir as mybir
    from concourse.tile import TileContext

    f32 = mybir.dt.float32
    bf16 = mybir.dt.bfloat16
    ALU = mybir.AluOpType
    ACT_COPY = mybir.ActivationFunctionType.Copy
    lam_eq = all(float(v) == float(lam4[0]) for v in lam4)
    lam0 = float(lam4[0])

    nc = bacc.Bacc("TRN2", target_bir_lowering=False, debug=False,
                   num_devices=N_CORES, detect_race_conditions=False)
    x = nc.dram_tensor("x", (PB, C, H, W), f32, kind="ExternalInput")
    out = nc.dram_tensor("out", (PB, 2, HO, WO), f32, kind="ExternalOutput")

    with TileContext(nc) as tc:
        with (
            tc.tile_pool(name="consts", bufs=1) as c_pool,
            tc.tile_pool(name="rhs", bufs=5) as rhs_pool,
            tc.tile_pool(name="work", bufs=1) as w_pool,
            tc.tile_pool(name="hmaps", bufs=2) as h_pool,
            tc.tile_pool(name="psum", bufs=2, space="PSUM") as ps_pool,
            tc.tile_pool(name="outs", bufs=3) as out_pool,
        ):
            # ---- one-time shift weights [128 window rows, BLK out rows] ----
            # window row w holds x row rlo+w; out row i0+m needs x rows
            # i0+m-1 (A/C) and i0+m-2, i0+m (B/D), i.e. w = m+off+k for
            # k in {-1} / {-2, 0} with off = i0-rlo.  Out-of-image rows are
            # simply not selected (top block: off=0; tail: mask w > wmax).
            with tc.tile_pool(name="scratch", bufs=1) as sc_pool:
                R2 = sc_pool.tile([128, BLK], f32, tag="R2")     # w + 2
                nc.gpsimd.iota(R2[:, :], pattern=[[0, BLK]], base=2,
                               channel_multiplier=1,
                               allow_small_or_imprecise_dtypes=True)
                Sm = {}
                for b in (-2, -1, 0, 1, 2):                  # m + b + 2 >= 0
                    t_ = sc_pool.tile([128, BLK], f32, tag=f"Sm{b}",
                                      name=f"Sm{b}")
                    nc.gpsimd.iota(t_[:, :], pattern=[[1, BLK]], base=b + 2,
                                   channel_multiplier=0,
                                   allow_small_or_imprecise_dtypes=True)
                    Sm[b] = t_
                e = {}
                for b in (-2, -1, 0, 1, 2):
                    t_ = sc_pool.tile([128, BLK], f32, tag=f"e{b}",
                                      name=f"e{b}")
                    nc.vector.tensor_tensor(t_[:, :], R2[:, :], Sm[b][:, :],
                                            ALU.is_equal)
                    e[b] = t_

                def mk(tag, pos, neg=None, scale=1.0, mask=None):
                    t_ = c_pool.tile([128, BLK], bf16, tag=tag, name=tag)
                    if neg is None:
                        nc.vector.tensor_scalar_mul(t_[:, :], e[pos][:, :],
                                                    scale)
                    else:
                        nc.vector.tensor_tensor(t_[:, :], e[pos][:, :],
                                                e[neg][:, :], ALU.subtract)
                    if mask is not None:
                        nc.vector.tensor_tensor(t_[:, :], t_[:, :],
                                                mask[:, :], ALU.mult)
                    return t_

                lamw = lam0 if lam_eq else 1.0
                # interior blocks (off=2): A w=m+1; B w=m (+) / m+2 (-)
                SlamI = mk("SlamI", 1, scale=lamw)
                S1I = mk("S1I", 1)
                SbdI = mk("SbdI", 0, 2)
                # top block (off=0): A w=m-1; B w=m-2 (+) / m (-)
                SlamT = mk("SlamT", -1, scale=lamw)
                S1T = mk("S1T", -1)
                SbdT = mk("SbdT", -2, 0)
                # tail block (off=2, only w<=9 valid): masked interior weights
                msk = sc_pool.tile([128, BLK], f32, tag="msk")
                nc.vector.tensor_scalar(msk[:, :], R2[:, :], 11.5, None,
                                        ALU.is_lt)
                SlamZ = mk("SlamZ", 1, scale=lamw, mask=msk)
                S1Z = mk("S1Z", 1, mask=msk)
                SbdZ = mk("SbdZ", 0, 2, mask=msk)

            # ---- work tiles: s1/mpAC are DVE-private (bufs=1), hAC/hBD are
            # read by the PE so they rotate over 2 buffers (otherwise the
            # next block's DVE writes stall on the previous block's matmuls)
            # bf16 intermediates: DVE 16-bit ops run ~2x, and the PE needs
            # bf16 operands anyway; costs ~1 extra rounding step per map
            s1 = w_pool.tile([128, 4 * 1024], bf16, tag="s1")
            mpAC = w_pool.tile([128, 2 * 516], bf16, tag="mpAC")
            if not lam_eq:
                tA = w_pool.tile([128, 4 * 512], f32, tag="tA")
            s1v = s1[:, :].rearrange("p (g k) -> p g k", k=1024)
            mpv = mpAC[:, :].rearrange("p (m c) -> p m c", c=516)
            nc.vector.memset(mpAC[:, :], 0.0)

            # zero both rotating buffers once: covers the column pads (data
            # ops never write them: mpAC data cols [2,514), hBD [1,513)) and
            # makes the never-selected stale rows of the first blocks finite
            for _ in range(2):
                hAC = h_pool.tile([128, 2 * WO], bf16, tag="hAC")
                hBD = h_pool.tile([128, 2 * WO], bf16, tag="hBD")
                nc.vector.memset(hAC[:, :], 0.0)
                nc.vector.memset(hBD[:, :], 0.0)

            # ---- main loop (tiny tail block first: the kernel's end is
            # input-stream-end + the last block's compute chain, so the
            # last-processed block should be a full one, not full + tail) --
            ORDER = [len(BLOCKS) - 1] + list(range(len(BLOCKS) - 1))
            for n in range(PB):
                for bi in ORDER:
                    i0, nr = BLOCKS[bi]
                    rlo = max(i0 - 2, 0)
                    rhi = min(i0 + nr, H)
                    P = rhi - rlo               # valid window rows
                    if bi == 0:
                        Sl, S1_, Sb = SlamT, S1T, SbdT
                    elif rhi == H and i0 + nr > H:
                        Sl, S1_, Sb = SlamZ, S1Z, SbdZ
                    else:
                        Sl, S1_, Sb = SlamI, S1I, SbdI
                    t = rhs_pool.tile([128, 16 * 512], f32, tag="rhs")
                    tv = t[:, :].rearrange("p (c w) -> p c w", w=512)
                    # ONE dma_start per block on the SP queue: [row, ch]
                    # descriptor order keeps DMA engine c streaming channel
                    # c's rows sequentially from HBM.  A single pure stream
                    # measures ~107 ns/2KB packet; splitting the load across
                    # queues (by channel, row, or via SWDGE) interleaves
                    # streams at the engines and degrades packets 25-60%.
                    nc.sync.dma_start(out=tv[0:P, :, :],
                                      in_=x[n, :, rlo:rhi, :].rearrange(
                                          "c r w -> r c w"))

                    hAC = h_pool.tile([128, 2 * WO], bf16, tag="hAC")
                    hBD = h_pool.tile([128, 2 * WO], bf16, tag="hBD")
                    hBDv = hBD[:, :].rearrange("p (m c) -> p m c", c=WO)
                    tg = t[:, :].rearrange("p (g k) -> p g k", k=2048)
                    # -- channel sums (fp32, all on DVE: other engines
                    #    contend for SBUF ports and slow everything down) --
                    if lam_eq:
                        nc.vector.tensor_tensor(
                            s1[0:P, :], tg[0:P, 0:4, 0:1024],
                            tg[0:P, 0:4, 1024:2048], ALU.add)
                    else:
                        tAv = tA[:, :].rearrange("p (c w) -> p c w", w=512)
                        for c4 in range(4):
                            nc.vector.tensor_scalar_mul(
                                tAv[0:P, c4, :], tv[0:P, c4, :], float(lam4[c4]))
                        nc.vector.tensor_tensor(
                            s1[0:P, 0:1024], tA[0:P, 0:1024],
                            tA[0:P, 1024:2048], ALU.add)
                        nc.vector.tensor_tensor(
                            s1v[0:P, 1:4, :], tg[0:P, 1:4, 0:1024],
                            tg[0:P, 1:4, 1024:2048], ALU.add)
                    # A,C sums into padded fp32 maps (s1 groups 0,2)
                    nc.vector.tensor_tensor(
                        mpv[0:P, 0:2, 2:514], s1v[0:P, 0:3:2, 0:512],
                        s1v[0:P, 0:3:2, 512:1024], ALU.add)
                    # horizontal diff -> bf16: hA[p,j] = Asum[p,j]-Asum[p,j-2]
                    hACv = hAC[:, :].rearrange("p (m c) -> p m c", c=WO)
                    nc.vector.tensor_tensor(
                        hACv[0:P, 0:2, :], mpv[0:P, 0:2, 2:516],
                        mpv[0:P, 0:2, 0:514], ALU.subtract)
                    # B,D sums straight into padded bf16 maps (s1 groups 1,3)
                    nc.vector.tensor_tensor(
                        hBDv[0:P, 0:2, 1:513], s1v[0:P, 1:4:2, 0:512],
                        s1v[0:P, 1:4:2, 512:1024], ALU.add)

                    # -- vertical shifts + combine on the PE: both stencil
                    #    terms accumulate into the same PSUM region; each
                    #    start->stop pair kept contiguous in issue order
                    #    (interleaved accumulation groups corrupt) --
                    psO1 = ps_pool.tile([128, 512], f32, tag="psO1", name="psO1")
                    psO2 = ps_pool.tile([128, 512], f32, tag="psO2", name="psO2")
                    psE = ps_pool.tile([128, 4], f32, tag="psE", name="psE")
                    nc.tensor.matmul(psO1[0:nr, :], Sl[:, 0:nr],
                                     hAC[:, 0:512], start=True, stop=False)
                    nc.tensor.matmul(psO1[0:nr, :], Sb[:, 0:nr],
                                     hBD[:, 0:512], start=False, stop=True)
                    nc.tensor.matmul(psE[0:nr, 0:2], Sl[:, 0:nr],
                                     hAC[:, 512:514], start=True, stop=False)
                    nc.tensor.matmul(psE[0:nr, 0:2], Sb[:, 0:nr],
                                     hBD[:, 512:514], start=False, stop=True)
                    nc.tensor.matmul(psO2[0:nr, :], S1_[:, 0:nr],
                                     hAC[:, WO:WO + 512], start=True, stop=False)
                    nc.tensor.matmul(psO2[0:nr, :], Sb[:, 0:nr],
                                     hBD[:, WO:WO + 512], start=False, stop=True)
                    nc.tensor.matmul(psE[0:nr, 2:4], S1_[:, 0:nr],
                                     hAC[:, WO + 512:WO + 514],
                                     start=True, stop=False)
                    nc.tensor.matmul(psE[0:nr, 2:4], Sb[:, 0:nr],
                                     hBD[:, WO + 512:WO + 514],
                                     start=False, stop=True)

                    # -- ACT drains PSUM straight into the output tile --
                    o = out_pool.tile([128, 2 * WO], f32, tag="o")
                    nc.scalar.activation(o[0:nr, 0:512], psO1[0:nr, :],
                                         ACT_COPY)
                    nc.scalar.activation(o[0:nr, 512:514], psE[0:nr, 0:2],
                                         ACT_COPY)
                    nc.scalar.activation(o[0:nr, WO:WO + 512], psO2[0:nr, :],
                                         ACT_COPY)
                    nc.scalar.activation(o[0:nr, WO + 512:2 * WO],
                                         psE[0:nr, 2:4], ACT_COPY)
                    osrc = o[0:nr, :].rearrange("p (ch w) -> p ch w", w=WO)
                    ov = out[n].rearrange("ch r w -> r ch w")
                    nc.scalar.dma_start(out=ov[i0:i0 + nr, :, :], in_=osrc)
    nc.finalize()
    return nc


def _get_nc(lam4):
    key = tuple(float(v) for v in lam4)
    if key not in _cache:
        _cache[key] = _build(key)
    return _cache[key]


def _run(xs: np.ndarray, lam4, trace: bool = False, tmpdir=None):
    from concourse.bass_utils import run_bass_kernel_spmd

    nc = _get_nc(lam4)
    in_maps = [{"x": np.ascontiguousarray(xs[PB * c:PB * (c + 1)])}
               for c in range(N_CORES)]
    res = run_bass_kernel_spmd(nc, in_maps, list(range(N_CORES)),
                               trace=trace, tmpdir=tmpdir)
    full = np.concatenate([res.results[c]["out"] for c in range(N_CORES)], axis=0)
    return full, res


def kernel(x, lam1x, lam2x, lam1y, lam2y):
    x = np.ascontiguousarray(np.asarray(x, dtype=np.float32))
    assert x.shape == (N, C, H, W), x.shape
    lam4 = np.asarray(lam1x, dtype=np.float32).reshape(-1)
    assert lam4.shape == (4,), lam4.shape
    full, _ = _run(x, lam4)
    return full


# revision 33
# speedup vs baseline: 1.0339x; 1.0055x over previous
"""Trainium2 Bass kernel for nn_Divergence2d.

Math (from the reference):
  q = C//4 = 4 channel groups A=x[:, :4], B=x[:,4:8], C=x[:,8:12], D=x[:,12:16]
  With per-group channel sums  Asum(r,c) = sum_ch lam_ch x[ch, r, c]  (lam only
  for group A):

    out1[i,j] = lam*(Asum[i-1, j] - Asum[i-1, j-2]) + Bsum[i-2, j-1] - Bsum[i, j-1]
    out2[i,j] =     (Csum[i-1, j] - Csum[i-1, j-2]) + Dsum[i-2, j-1] - Dsum[i, j-1]

  for i,j in [0, 514), with zero padding outside [0,512).

Strategy (v5, pure data parallel, 2 images per core on 8 cores):
  The op is memory-bound: 38 MB HBM traffic/core across 19k 2 KB DMA
  descriptors is a ~125 us floor at the measured per-engine packet rate.
  Every design choice below exists to keep the 16 DMA engines streaming:

  - per 126-row block, a 128-row window of all 16 channels is loaded into
    [128 rows, 16ch x 512].  Descriptor order [row, ch] makes DMA engine c
    stream channel c's rows sequentially from HBM (HBM-friendly).  The load
    is split into two row-halves issued on different queues (SP + Pool
    SWDGE) because a HWDGE dma_start occupies its queue for the whole
    transfer - one queue would serialize input delivery at ~11 us/block.
  - DVE: channel sums via grouped adds (3D APs, fp32 in / bf16 out), then
    the *horizontal* stencil diff, writing bf16 maps
    (hA = Asum[:, j] - Asum[:, j-2]; hB = Bsum[:, j-1] via padded layout).
    DVE instruction cost is ~1.08 ns (fp32) / ~0.67 ns (bf16) per free-dim
    element regardless of partition count, so ops are organized as few
    wide-partition passes and intermediates are bf16.
  - TensorE (bf16, full rate): the *vertical* shifts as one-hot shift
    matmuls, with the final combine done by PSUM accumulation:
    psOut1 = (lam*S1)@hA + Sbd@hB (start/stop pairs kept contiguous -
    interleaving accumulation groups corrupts results).  Compute-engine
    APs cannot start at a nonzero partition on TRN2, so row shifts must
    go through the PE.  Image-boundary zero padding is folded into
    per-block weight variants (top block shifts by -1, tail block masks
    out-of-range rows) so no window/map memsets are needed at all.
  - ACT drains PSUM straight into the output tile; one store per block on
    the ACT queue (dependency-aligned with the drains).

  Only the maps are bf16-rounded (weights are exact 0/+-lam/+-1), giving
  ~2.7e-3 l2 rel error vs the 2e-2 gate.  Measured: ~144-146 us vs the
  173-181 us fp32-matmul baseline and a ~125 us DMA-engine floor.
"""
import sys

for _p in (
    "/root/.axon_site",
    "/root/.axon_site/_ro/trn_rl_repo",
    "/root/.axon_site/_ro/pypackages",
    "/opt/trn_rl_repo",
):
    if _p not in sys.path:
        sys.path.append(_p)

import numpy as np

N_CORES = 8
N, C, H, W = 16, 16, 512, 512
PB = N // N_CORES          # images per core
HO = WO = H + 2            # 514
BLK = 126                  # output rows per block
BLOCKS = []
_i0 = 0
while _i0 < HO:
    BLOCKS.append((_i0, min(BLK, HO - _i0)))
    _i0 += BLK
# -> [(0,126), (126,126), (252,126), (378,126), (504,10)]

_cache = {}


def _build(lam4):
    import concourse.bacc as bacc
    import concourse.mybir as mybir
    from concourse.tile import TileContext

    f32 = mybir.dt.float32
    bf16 = mybir.dt.bfloat16
    ALU = mybir.AluOpType
    ACT_COPY = mybir.ActivationFunctionType.Copy
    lam_eq = all(float(v) == float(lam4[0]) for v in lam4)
    lam0 = float(lam4[0])

    nc = bacc.Bacc("TRN2", target_bir_lowering=False, debug=False,
                   num_devices=N_CORES, detect_race_conditions=False)
    x = nc.dram_tensor("x", (PB, C, H, W), f32, kind="ExternalInput")
    out = nc.dram_tensor("out", (PB, 2, HO, WO), f32, kind="ExternalOutput")

    with TileContext(nc) as tc:
        with (
            tc.tile_pool(name="consts", bufs=1) as c_pool,
            tc.tile_pool(name="rhs", bufs=5) as rhs_pool,
            tc.tile_pool(name="work", bufs=1) as w_pool,
            tc.tile_pool(name="hmaps", bufs=2) as h_pool,
            tc.tile_pool(name="psum", bufs=2, space="PSUM") as ps_pool,
            tc.tile_pool(name="outs", bufs=3) as out_pool,
        ):
            # ---- one-time shift weights [128 window rows, BLK out rows] ----
            # window row w holds x row rlo+w; out row i0+m needs x rows
            # i0+m-1 (A/C) and i0+m-2, i0+m (B/D), i.e. w = m+off+k for
            # k in {-1} / {-2, 0} with off = i0-rlo.  Out-of-image rows are
            # simply not selected (top block: off=0; tail: mask w > wmax).
            with tc.tile_pool(name="scratch", bufs=1) as sc_pool:
                R2 = sc_pool.tile([128, BLK], f32, tag="R2")     # w + 2
                nc.gpsimd.iota(R2[:, :], pattern=[[0, BLK]], base=2,
                               channel_multiplier=1,
                               allow_small_or_imprecise_dtypes=True)
                Sm = {}
                for b in (-2, -1, 0, 1, 2):                  # m + b + 2 >= 0
                    t_ = sc_pool.tile([128, BLK], f32, tag=f"Sm{b}",
                                      name=f"Sm{b}")
                    nc.gpsimd.iota(t_[:, :], pattern=[[1, BLK]], base=b + 2,
                                   channel_multiplier=0,
                                   allow_small_or_imprecise_dtypes=True)
                    Sm[b] = t_
                e = {}
                for b in (-2, -1, 0, 1, 2):
                    t_ = sc_pool.tile([128, BLK], f32, tag=f"e{b}",
                                      name=f"e{b}")
                    nc.vector.tensor_tensor(t_[:, :], R2[:, :], Sm[b][:, :],
                                            ALU.is_equal)
                    e[b] = t_

                def mk(tag, pos, neg=None, scale=1.0, mask=None):
                    t_ = c_pool.tile([128, BLK], bf16, tag=tag, name=tag)
                    if neg is None:
                        nc.vector.tensor_scalar_mul(t_[:, :], e[pos][:, :],
                                                    scale)
                    else:
                        nc.vector.tensor_tensor(t_[:, :], e[pos][:, :],
                                                e[neg][:, :], ALU.subtract)
                    if mask is not None:
                        nc.vector.tensor_tensor(t_[:, :], t_[:, :],
                                                mask[:, :], ALU.mult)
                    return t_

                lamw = lam0 if lam_eq else 1.0
                # interior blocks (off=2): A w=m+1; B w=m (+) / m+2 (-)
                SlamI = mk("SlamI", 1, scale=lamw)
                S1I = mk("S1I", 1)
                SbdI = mk("SbdI", 0, 2)
                # top block (off=0): A w=m-1; B w=m-2 (+) / m (-)
                SlamT = mk("SlamT", -1, scale=lamw)
                S1T = mk("S1T", -1)
                SbdT = mk("SbdT", -2, 0)
                # tail block (off=2, only w<=9 valid): masked interior weights
                msk = sc_pool.tile([128, BLK], f32, tag="msk")
                nc.vector.tensor_scalar(msk[:, :], R2[:, :], 11.5, None,
                                        ALU.is_lt)
                SlamZ = mk("SlamZ", 1, scale=lamw, mask=msk)
                S1Z = mk("S1Z", 1, mask=msk)
                SbdZ = mk("SbdZ", 0, 2, mask=msk)

            # ---- work tiles: s1/mpAC are DVE-private (bufs=1), hAC/hBD are
            # read by the PE so they rotate over 2 buffers (otherwise the
            # next block's DVE writes stall on the previous block's matmuls)
            # bf16 intermediates: DVE 16-bit ops run ~2x, and the PE needs
            # bf16 operands anyway; costs ~1 extra rounding step per map
            s1 = w_pool.tile([128, 4 * 1024], bf16, tag="s1")
            mpAC = w_pool.tile([128, 2 * 516], bf16, tag="mpAC")
            if not lam_eq:
                tA = w_pool.tile([128, 4 * 512], f32, tag="tA")
            s1v = s1[:, :].rearrange("p (g k) -> p g k", k=1024)
            mpv = mpAC[:, :].rearrange("p (m c) -> p m c", c=516)
            nc.vector.memset(mpAC[:, :], 0.0)

            # zero both rotating buffers once: covers the column pads (data
            # ops never write them: mpAC data cols [2,514), hBD [1,513)) and
            # makes the never-selected stale rows of the first blocks finite
            for _ in range(2):
                hAC = h_pool.tile([128, 2 * WO], bf16, tag="hAC")
                hBD = h_pool.tile([128, 2 * WO], bf16, tag="hBD")
                nc.vector.memset(hAC[:, :], 0.0)
                nc.vector.memset(hBD[:, :], 0.0)

            # ---- main loop (tiny tail block first: the kernel's end is
            # input-stream-end + the last block's compute chain, so the
            # last-processed block should be a full one, not full + tail) --
            ORDER = [len(BLOCKS) - 1] + list(range(len(BLOCKS) - 1))
            for n in range(PB):
                for bi in ORDER:
                    i0, nr = BLOCKS[bi]
                    rlo = max(i0 - 2, 0)
                    rhi = min(i0 + nr, H)
                    P = rhi - rlo               # valid window rows
                    if bi == 0:
                        Sl, S1_, Sb = SlamT, S1T, SbdT
                    elif rhi == H and i0 + nr > H:
                        Sl, S1_, Sb = SlamZ, S1Z, SbdZ
                    else:
                        Sl, S1_, Sb = SlamI, S1I, SbdI
                    t = rhs_pool.tile([128, 16 * 512], f32, tag="rhs")
                    tv = t[:, :].rearrange("p (c w) -> p c w", w=512)
                    # ONE dma_start per block on the SP queue: [row, ch]
                    # descriptor order keeps DMA engine c streaming channel
                    # c's rows sequentially from HBM.  A single pure stream
                    # measures ~107 ns/2KB packet; splitting the load across
                    # queues (by channel, row, or via SWDGE) interleaves
                    # streams at the engines and degrades packets 25-60%.
                    nc.sync.dma_start(out=tv[0:P, :, :],
                                      in_=x[n, :, rlo:rhi, :].rearrange(
                                          "c r w -> r c w"))

                    hAC = h_pool.tile([128, 2 * WO], bf16, tag="hAC")
                    hBD = h_pool.tile([128, 2 * WO], bf16, tag="hBD")
                    hBDv = hBD[:, :].rearrange("p (m c) -> p m c", c=WO)
                    tg = t[:, :].rearrange("p (g k) -> p g k", k=2048)
                    # -- channel sums (fp32 in, bf16 out; all on DVE: other
                    #    engines contend for SBUF ports and slow both down) --
                    if lam_eq:
                        nc.vector.tensor_tensor(
                            s1[0:P, :], tg[0:P, 0:4, 0:1024],
                            tg[0:P, 0:4, 1024:2048], ALU.add)
                    else:
                        tAv = tA[:, :].rearrange("p (c w) -> p c w", w=512)
                        for c4 in range(4):
                            nc.vector.tensor_scalar_mul(
                                tAv[0:P, c4, :], tv[0:P, c4, :], float(lam4[c4]))
                        nc.vector.tensor_tensor(
                            s1[0:P, 0:1024], tA[0:P, 0:1024],
                            tA[0:P, 1024:2048], ALU.add)
                        nc.vector.tensor_tensor(
                            s1v[0:P, 1:4, :], tg[0:P, 1:4, 0:1024],
                            tg[0:P, 1:4, 1024:2048], ALU.add)
                    # A,C sums into padded fp32 maps (s1 groups 0,2)
                    nc.vector.tensor_tensor(
                        mpv[0:P, 0:2, 2:514], s1v[0:P, 0:3:2, 0:512],
                        s1v[0:P, 0:3:2, 512:1024], ALU.add)
                    # horizontal diff -> bf16: hA[p,j] = Asum[p,j]-Asum[p,j-2]
                    hACv = hAC[:, :].rearrange("p (m c) -> p m c", c=WO)
                    nc.vector.tensor_tensor(
                        hACv[0:P, 0:2, :], mpv[0:P, 0:2, 2:516],
                        mpv[0:P, 0:2, 0:514], ALU.subtract)
                    # B,D sums straight into padded bf16 maps (s1 groups 1,3)
                    nc.vector.tensor_tensor(
                        hBDv[0:P, 0:2, 1:513], s1v[0:P, 1:4:2, 0:512],
                        s1v[0:P, 1:4:2, 512:1024], ALU.add)

                    # -- vertical shifts + combine on the PE: both stencil
                    #    terms accumulate into the same PSUM region; each
                    #    start->stop pair kept contiguous in issue order
                    #    (interleaved accumulation groups corrupt) --
                    psO1 = ps_pool.tile([128, 512], f32, tag="psO1", name="psO1")
                    psO2 = ps_pool.tile([128, 512], f32, tag="psO2", name="psO2")
                    psE = ps_pool.tile([128, 4], f32, tag="psE", name="psE")
                    nc.tensor.matmul(psO1[0:nr, :], Sl[:, 0:nr],
                                     hAC[:, 0:512], start=True, stop=False)
                    nc.tensor.matmul(psO1[0:nr, :], Sb[:, 0:nr],
                                     hBD[:, 0:512], start=False, stop=True)
                    nc.tensor.matmul(psE[0:nr, 0:2], Sl[:, 0:nr],
                                     hAC[:, 512:514], start=True, stop=False)
                    nc.tensor.matmul(psE[0:nr, 0:2], Sb[:, 0:nr],
                                     hBD[:, 512:514], start=False, stop=True)
                    nc.tensor.matmul(psO2[0:nr, :], S1_[:, 0:nr],
                                     hAC[:, WO:WO + 512], start=True, stop=False)
                    nc.tensor.matmul(psO2[0:nr, :], Sb[:, 0:nr],
                                     hBD[:, WO:WO + 512], start=False, stop=True)
                    nc.tensor.matmul(psE[0:nr, 2:4], S1_[:, 0:nr],
                                     hAC[:, WO + 512:WO + 514],
                                     start=True, stop=False)
                    nc.tensor.matmul(psE[0:nr, 2:4], Sb[:, 0:nr],
                                     hBD[:, WO + 512:WO + 514],
                                     start=False, stop=True)

                    # -- ACT drains PSUM straight into the output tile --
                    o = out_pool.tile([128, 2 * WO], f32, tag="o")
                    nc.scalar.activation(o[0:nr, 0:512], psO1[0:nr, :],
                                         ACT_COPY)
                    nc.scalar.activation(o[0:nr, 512:514], psE[0:nr, 0:2],
                                         ACT_COPY)
                    nc.scalar.activation(o[0:nr, WO:WO + 512], psO2[0:nr, :],
                                         ACT_COPY)
                    nc.scalar.activation(o[0:nr, WO + 512:2 * WO],
                                         psE[0:nr, 2:4], ACT_COPY)
                    osrc = o[0:nr, :].rearrange("p (ch w) -> p ch w", w=WO)
                    ov = out[n].rearrange("ch r w -> r ch w")
                    nc.scalar.dma_start(out=ov[i0:i0 + nr, :, :], in_=osrc)
    nc.finalize()
    return nc


def _get_nc(lam4):
    key = tuple(float(v) for v in lam4)
    if key not in _cache:
        _cache[key] = _build(key)
    return _cache[key]


def _run(xs: np.ndarray, lam4, trace: bool = False, tmpdir=None):
    from concourse.bass_utils import run_bass_kernel_spmd

    nc = _get_nc(lam4)
    in_maps = [{"x": np.ascontiguousarray(xs[PB * c:PB * (c + 1)])}
               for c in range(N_CORES)]
    res = run_bass_kernel_spmd(nc, in_maps, list(range(N_CORES)),
                               trace=trace, tmpdir=tmpdir)
    full = np.concatenate([res.results[c]["out"] for c in range(N_CORES)], axis=0)
    return full, res


def kernel(x, lam1x, lam2x, lam1y, lam2y):
    x = np.ascontiguousarray(np.asarray(x, dtype=np.float32))
    assert x.shape == (N, C, H, W), x.shape
    lam4 = np.asarray(lam1x, dtype=np.float32).reshape(-1)
    assert lam4.shape == (4,), lam4.shape
    full, _ = _run(x, lam4)
    return full


# revision 34
# speedup vs baseline: 1.0698x; 1.0348x over previous
"""Trainium2 Bass kernel for nn_Divergence2d.

Math (from the reference):
  q = C//4 = 4 channel groups A=x[:, :4], B=x[:,4:8], C=x[:,8:12], D=x[:,12:16]
  With per-group channel sums  Asum(r,c) = sum_ch lam_ch x[ch, r, c]  (lam only
  for group A):

    out1[i,j] = lam*(Asum[i-1, j] - Asum[i-1, j-2]) + Bsum[i-2, j-1] - Bsum[i, j-1]
    out2[i,j] =     (Csum[i-1, j] - Csum[i-1, j-2]) + Dsum[i-2, j-1] - Dsum[i, j-1]

  for i,j in [0, 514), with zero padding outside [0,512).

Strategy (v5, pure data parallel, 2 images per core on 8 cores):
  The op is memory-bound: 38 MB HBM traffic/core across 19k 2 KB DMA
  descriptors is a ~125 us floor at the measured per-engine packet rate.
  Every design choice below exists to keep the 16 DMA engines streaming:

  - per 126-row block, a 128-row window of all 16 channels is loaded into
    [128 rows, 16ch x 512].  Descriptor order [row, ch] makes DMA engine c
    stream channel c's rows sequentially from HBM (HBM-friendly).  The load
    is split into two row-halves issued on different queues (SP + Pool
    SWDGE) because a HWDGE dma_start occupies its queue for the whole
    transfer - one queue would serialize input delivery at ~11 us/block.
  - DVE: channel sums via grouped adds (3D APs, fp32 in / bf16 out), then
    the *horizontal* stencil diff, writing bf16 maps
    (hA = Asum[:, j] - Asum[:, j-2]; hB = Bsum[:, j-1] via padded layout).
    DVE instruction cost is ~1.08 ns (fp32) / ~0.67 ns (bf16) per free-dim
    element regardless of partition count, so ops are organized as few
    wide-partition passes and intermediates are bf16.
  - TensorE (bf16, full rate): the *vertical* shifts as one-hot shift
    matmuls, with the final combine done by PSUM accumulation:
    psOut1 = (lam*S1)@hA + Sbd@hB (start/stop pairs kept contiguous -
    interleaving accumulation groups corrupts results).  Compute-engine
    APs cannot start at a nonzero partition on TRN2, so row shifts must
    go through the PE.  Image-boundary zero padding is folded into
    per-block weight variants (top block shifts by -1, tail block masks
    out-of-range rows) so no window/map memsets are needed at all.
  - ACT drains PSUM straight into the output tile; one store per block on
    the ACT queue (dependency-aligned with the drains).

  Only the maps are bf16-rounded (weights are exact 0/+-lam/+-1), giving
  ~2.7e-3 l2 rel error vs the 2e-2 gate.  Measured: ~144-146 us vs the
  173-181 us fp32-matmul baseline and a ~125 us DMA-engine floor.
"""
import sys

for _p in (
    "/root/.axon_site",
    "/root/.axon_site/_ro/trn_rl_repo",
    "/root/.axon_site/_ro/pypackages",
    "/opt/trn_rl_repo",
):
    if _p not in sys.path:
        sys.path.append(_p)

import numpy as np

N_CORES = 8
N, C, H, W = 16, 16, 512, 512
PB = N // N_CORES          # images per core
HO = WO = H + 2            # 514
BLK = 126                  # output rows per block
BLOCKS = []
_i0 = 0
while _i0 < HO:
    BLOCKS.append((_i0, min(BLK, HO - _i0)))
    _i0 += BLK
# -> [(0,126), (126,126), (252,126), (378,126), (504,10)]

_cache = {}


def _build(lam4):
    import concourse.bacc as bacc
    import concourse.mybir as mybir
    from concourse.tile import TileContext

    f32 = mybir.dt.float32
    bf16 = mybir.dt.bfloat16
    ALU = mybir.AluOpType
    ACT_COPY = mybir.ActivationFunctionType.Copy
    lam_eq = all(float(v) == float(lam4[0]) for v in lam4)
    lam0 = float(lam4[0])

    nc = bacc.Bacc("TRN2", target_bir_lowering=False, debug=False,
                   num_devices=N_CORES, detect_race_conditions=False)
    x = nc.dram_tensor("x", (PB, C, H, W), f32, kind="ExternalInput")
    out = nc.dram_tensor("out", (PB, 2, HO, WO), f32, kind="ExternalOutput")

    with TileContext(nc) as tc:
        with (
            tc.tile_pool(name="consts", bufs=1) as c_pool,
            tc.tile_pool(name="rhs", bufs=5) as rhs_pool,
            tc.tile_pool(name="work", bufs=1) as w_pool,
            tc.tile_pool(name="hmaps", bufs=3) as h_pool,
            tc.tile_pool(name="psum", bufs=2, space="PSUM") as ps_pool,
            tc.tile_pool(name="outs", bufs=4) as out_pool,
        ):
            # ---- one-time shift weights [128 window rows, BLK out rows] ----
            # window row w holds x row rlo+w; out row i0+m needs x rows
            # i0+m-1 (A/C) and i0+m-2, i0+m (B/D), i.e. w = m+off+k for
            # k in {-1} / {-2, 0} with off = i0-rlo.  Out-of-image rows are
            # simply not selected (top block: off=0; tail: mask w > wmax).
            with tc.tile_pool(name="scratch", bufs=1) as sc_pool:
                R2 = sc_pool.tile([128, BLK], f32, tag="R2")     # w + 2
                nc.gpsimd.iota(R2[:, :], pattern=[[0, BLK]], base=2,
                               channel_multiplier=1,
                               allow_small_or_imprecise_dtypes=True)
                Sm = {}
                for b in (-2, -1, 0, 1, 2):                  # m + b + 2 >= 0
                    t_ = sc_pool.tile([128, BLK], f32, tag=f"Sm{b}",
                                      name=f"Sm{b}")
                    nc.gpsimd.iota(t_[:, :], pattern=[[1, BLK]], base=b + 2,
                                   channel_multiplier=0,
                                   allow_small_or_imprecise_dtypes=True)
                    Sm[b] = t_
                e = {}
                for b in (-2, -1, 0, 1, 2):
                    t_ = sc_pool.tile([128, BLK], f32, tag=f"e{b}",
                                      name=f"e{b}")
                    nc.vector.tensor_tensor(t_[:, :], R2[:, :], Sm[b][:, :],
                                            ALU.is_equal)
                    e[b] = t_

                def mk(tag, pos, neg=None, scale=1.0, mask=None):
                    t_ = c_pool.tile([128, BLK], bf16, tag=tag, name=tag)
                    if neg is None:
                        nc.vector.tensor_scalar_mul(t_[:, :], e[pos][:, :],
                                                    scale)
                    else:
                        nc.vector.tensor_tensor(t_[:, :], e[pos][:, :],
                                                e[neg][:, :], ALU.subtract)
                    if mask is not None:
                        nc.vector.tensor_tensor(t_[:, :], t_[:, :],
                                                mask[:, :], ALU.mult)
                    return t_

                lamw = lam0 if lam_eq else 1.0
                # interior blocks (off=2): A w=m+1; B w=m (+) / m+2 (-)
                SlamI = mk("SlamI", 1, scale=lamw)
                S1I = mk("S1I", 1)
                SbdI = mk("SbdI", 0, 2)
                # top block (off=0): A w=m-1; B w=m-2 (+) / m (-)
                SlamT = mk("SlamT", -1, scale=lamw)
                S1T = mk("S1T", -1)
                SbdT = mk("SbdT", -2, 0)
                # tail block (off=2, only w<=9 valid): masked interior weights
                msk = sc_pool.tile([128, BLK], f32, tag="msk")
                nc.vector.tensor_scalar(msk[:, :], R2[:, :], 11.5, None,
                                        ALU.is_lt)
                SlamZ = mk("SlamZ", 1, scale=lamw, mask=msk)
                S1Z = mk("S1Z", 1, mask=msk)
                SbdZ = mk("SbdZ", 0, 2, mask=msk)

            # ---- work tiles: s1/mpAC are DVE-private (bufs=1), hAC/hBD are
            # read by the PE so they rotate over 2 buffers (otherwise the
            # next block's DVE writes stall on the previous block's matmuls)
            # bf16 intermediates: DVE 16-bit ops run ~2x, and the PE needs
            # bf16 operands anyway; costs ~1 extra rounding step per map
            s1 = w_pool.tile([128, 4 * 1024], bf16, tag="s1")
            mpAC = w_pool.tile([128, 2 * 516], bf16, tag="mpAC")
            if not lam_eq:
                tA = w_pool.tile([128, 4 * 512], f32, tag="tA")
            s1v = s1[:, :].rearrange("p (g k) -> p g k", k=1024)
            mpv = mpAC[:, :].rearrange("p (m c) -> p m c", c=516)
            nc.vector.memset(mpAC[:, :], 0.0)

            # zero both rotating buffers once: covers the column pads (data
            # ops never write them: mpAC data cols [2,514), hBD [1,513)) and
            # makes the never-selected stale rows of the first blocks finite
            for _ in range(3):
                hAC = h_pool.tile([128, 2 * WO], bf16, tag="hAC")
                hBD = h_pool.tile([128, 2 * WO], bf16, tag="hBD")
                nc.vector.memset(hAC[:, :], 0.0)
                nc.vector.memset(hBD[:, :], 0.0)

            # ---- main loop (tiny tail block first: the kernel's end is
            # input-stream-end + the last block's compute chain, so the
            # last-processed block should be a full one, not full + tail) --
            ORDER = [len(BLOCKS) - 1] + list(range(len(BLOCKS) - 1))
            for n in range(PB):
                for bi in ORDER:
                    i0, nr = BLOCKS[bi]
                    rlo = max(i0 - 2, 0)
                    rhi = min(i0 + nr, H)
                    P = rhi - rlo               # valid window rows
                    if bi == 0:
                        Sl, S1_, Sb = SlamT, S1T, SbdT
                    elif rhi == H and i0 + nr > H:
                        Sl, S1_, Sb = SlamZ, S1Z, SbdZ
                    else:
                        Sl, S1_, Sb = SlamI, S1I, SbdI
                    t = rhs_pool.tile([128, 16 * 512], f32, tag="rhs")
                    tv = t[:, :].rearrange("p (c w) -> p c w", w=512)
                    # ONE dma_start per block on the SP queue: [row, ch]
                    # descriptor order keeps DMA engine c streaming channel
                    # c's rows sequentially from HBM.  A single pure stream
                    # measures ~107 ns/2KB packet; splitting the load across
                    # queues (by channel, row, or via SWDGE) interleaves
                    # streams at the engines and degrades packets 25-60%.
                    nc.sync.dma_start(out=tv[0:P, :, :],
                                      in_=x[n, :, rlo:rhi, :].rearrange(
                                          "c r w -> r c w"))

                    hAC = h_pool.tile([128, 2 * WO], bf16, tag="hAC")
                    hBD = h_pool.tile([128, 2 * WO], bf16, tag="hBD")
                    hBDv = hBD[:, :].rearrange("p (m c) -> p m c", c=WO)
                    tg = t[:, :].rearrange("p (g k) -> p g k", k=2048)
                    # -- channel sums (fp32 in, bf16 out; all on DVE: other
                    #    engines contend for SBUF ports and slow both down) --
                    if lam_eq:
                        nc.vector.tensor_tensor(
                            s1[0:P, :], tg[0:P, 0:4, 0:1024],
                            tg[0:P, 0:4, 1024:2048], ALU.add)
                    else:
                        tAv = tA[:, :].rearrange("p (c w) -> p c w", w=512)
                        for c4 in range(4):
                            nc.vector.tensor_scalar_mul(
                                tAv[0:P, c4, :], tv[0:P, c4, :], float(lam4[c4]))
                        nc.vector.tensor_tensor(
                            s1[0:P, 0:1024], tA[0:P, 0:1024],
                            tA[0:P, 1024:2048], ALU.add)
                        nc.vector.tensor_tensor(
                            s1v[0:P, 1:4, :], tg[0:P, 1:4, 0:1024],
                            tg[0:P, 1:4, 1024:2048], ALU.add)
                    # A,C sums into padded fp32 maps (s1 groups 0,2)
                    nc.vector.tensor_tensor(
                        mpv[0:P, 0:2, 2:514], s1v[0:P, 0:3:2, 0:512],
                        s1v[0:P, 0:3:2, 512:1024], ALU.add)
                    # horizontal diff -> bf16: hA[p,j] = Asum[p,j]-Asum[p,j-2]
                    hACv = hAC[:, :].rearrange("p (m c) -> p m c", c=WO)
                    nc.vector.tensor_tensor(
                        hACv[0:P, 0:2, :], mpv[0:P, 0:2, 2:516],
                        mpv[0:P, 0:2, 0:514], ALU.subtract)
                    # B,D sums straight into padded bf16 maps (s1 groups 1,3)
                    nc.vector.tensor_tensor(
                        hBDv[0:P, 0:2, 1:513], s1v[0:P, 1:4:2, 0:512],
                        s1v[0:P, 1:4:2, 512:1024], ALU.add)

                    # -- vertical shifts + combine on the PE: both stencil
                    #    terms accumulate into the same PSUM region; each
                    #    start->stop pair kept contiguous in issue order
                    #    (interleaved accumulation groups corrupt) --
                    psO1 = ps_pool.tile([128, 512], f32, tag="psO1", name="psO1")
                    psO2 = ps_pool.tile([128, 512], f32, tag="psO2", name="psO2")
                    psE = ps_pool.tile([128, 4], f32, tag="psE", name="psE")
                    nc.tensor.matmul(psO1[0:nr, :], Sl[:, 0:nr],
                                     hAC[:, 0:512], start=True, stop=False)
                    nc.tensor.matmul(psO1[0:nr, :], Sb[:, 0:nr],
                                     hBD[:, 0:512], start=False, stop=True)
                    nc.tensor.matmul(psE[0:nr, 0:2], Sl[:, 0:nr],
                                     hAC[:, 512:514], start=True, stop=False)
                    nc.tensor.matmul(psE[0:nr, 0:2], Sb[:, 0:nr],
                                     hBD[:, 512:514], start=False, stop=True)
                    nc.tensor.matmul(psO2[0:nr, :], S1_[:, 0:nr],
                                     hAC[:, WO:WO + 512], start=True, stop=False)
                    nc.tensor.matmul(psO2[0:nr, :], Sb[:, 0:nr],
                                     hBD[:, WO:WO + 512], start=False, stop=True)
                    nc.tensor.matmul(psE[0:nr, 2:4], S1_[:, 0:nr],
                                     hAC[:, WO + 512:WO + 514],
                                     start=True, stop=False)
                    nc.tensor.matmul(psE[0:nr, 2:4], Sb[:, 0:nr],
                                     hBD[:, WO + 512:WO + 514],
                                     start=False, stop=True)

                    # -- ACT drains PSUM straight into the output tile --
                    o = out_pool.tile([128, 2 * WO], f32, tag="o")
                    nc.scalar.activation(o[0:nr, 0:512], psO1[0:nr, :],
                                         ACT_COPY)
                    nc.scalar.activation(o[0:nr, 512:514], psE[0:nr, 0:2],
                                         ACT_COPY)
                    nc.scalar.activation(o[0:nr, WO:WO + 512], psO2[0:nr, :],
                                         ACT_COPY)
                    nc.scalar.activation(o[0:nr, WO + 512:2 * WO],
                                         psE[0:nr, 2:4], ACT_COPY)
                    osrc = o[0:nr, :].rearrange("p (ch w) -> p ch w", w=WO)
                    ov = out[n].rearrange("ch r w -> r ch w")
                    nc.scalar.dma_start(out=ov[i0:i0 + nr, :, :], in_=osrc)
    nc.finalize()
    return nc


def _get_nc(lam4):
    key = tuple(float(v) for v in lam4)
    if key not in _cache:
        _cache[key] = _build(key)
    return _cache[key]


def _run(xs: np.ndarray, lam4, trace: bool = False, tmpdir=None):
    from concourse.bass_utils import run_bass_kernel_spmd

    nc = _get_nc(lam4)
    in_maps = [{"x": np.ascontiguousarray(xs[PB * c:PB * (c + 1)])}
               for c in range(N_CORES)]
    res = run_bass_kernel_spmd(nc, in_maps, list(range(N_CORES)),
                               trace=trace, tmpdir=tmpdir)
    full = np.concatenate([res.results[c]["out"] for c in range(N_CORES)], axis=0)
    return full, res


def kernel(x, lam1x, lam2x, lam1y, lam2y):
    x = np.ascontiguousarray(np.asarray(x, dtype=np.float32))
    assert x.shape == (N, C, H, W), x.shape
    lam4 = np.asarray(lam1x, dtype=np.float32).reshape(-1)
    assert lam4.shape == (4,), lam4.shape
    full, _ = _run(x, lam4)
    return full
